# revision 64
# baseline (speedup 1.0000x reference)
"""Trainium2 Bass kernel for nn_MemoryModel (scatter_memory, 8 cores).

Math (per stage): the 8-point Gauss-Legendre quadrature over matrix
polynomials collapses algebraically:

  LHS_k = I - REG*t_k*D + REG^2*(t_k*D@L + t_k^2/2*D@D)      (D=delta_L, L=L_agg)
  integral = sum_k (LHS_k @ (w_k*V)) * exp(dA*t_k)
           = V*S0 - REG*U*S1 + REG^2*P*S1 + REG^2/2*Q*S2
  with V = X - REG*(L@X),  U = D@V, W1 = L@V, P = D@W1, Q = D@U
  and moments S_j = sum_k w_k t_k^j exp(dA t_k)   (elementwise [n,H])
  As_bar @ M = M - REG*(D@M) + REG^2*(D@(L@M)) + REG^2/2*(D@(D@M))

So each stage costs 9 matmuls of [1024,1024]@[1024,16] per core instead of
nine n^3 products; no n^3 matmul anywhere.

Sharding: H=128 is column-sharded 8 ways (16 cols/core). The [1024,1024]
operators (as transposed, k-tile-packed bf16 hi/lo splits) are replicated;
the per-node small pipeline runs in "transposed land" (H on partitions)
replicated on every core; heavy chains run per-core on the 16-column shard
in node-packed layout [128p, 8q, 16h] (node = 128q+p). The memory tables
m1/m2 are column-sharded [100000,16] per core and gathered on-device with
indirect DMA. One AllGather ([16,1024] -> [128,1024]) carries stage-1
output c1^T to all cores for stage 2.
"""
import os
import sys

import numpy as np

for _p in ("/opt/trn_rl_repo", "/root/.axon_site/_ro/trn_rl_repo"):
    if os.path.isdir(_p) and _p not in sys.path:
        sys.path.insert(0, _p)

import ml_dtypes  # noqa: E402
import concourse.bass as bass  # noqa: E402
import concourse.bacc as bacc  # noqa: E402
import concourse.mybir as mybir  # noqa: E402
import concourse.tile as tile  # noqa: E402
from concourse.bass_utils import run_bass_kernel_spmd  # noqa: E402

F32 = mybir.dt.float32
BF16 = mybir.dt.bfloat16
I32 = mybir.dt.int32
AF = mybir.ActivationFunctionType
OP = mybir.AluOpType
BF = ml_dtypes.bfloat16

NA, H, DIN, E, NN, ED = 1024, 128, 172, 256, 100000, 1
KD = DIN + 2 * ED  # 174
REG = 0.1
REG2 = REG * REG
NCORES = 8
HS = 16  # H columns per core
NQ = 8  # node tiles (1024/128)

_gl_nodes = [-0.1834346424956498, -0.525532409916329, -0.7966664774136267,
             -0.9602898564975363, 0.1834346424956498, 0.525532409916329,
             0.7966664774136267, 0.9602898564975363]
_gl_w = [0.362683783378362, 0.3137066458778873, 0.2223810344533745,
         0.1012285362903763] * 2
T_NODES = [0.5 * (x + 1.0) for x in _gl_nodes]
T_W = [0.5 * w for w in _gl_w]

SPLIT_FIRST = False  # hi/lo bf16 split for the first-order passes (L1, D1)

_BUILD_CACHE = {}


def _pin_act_table_set():
    """Restrict walrus's ACT-table choice to natural_log_exp_and_others so
    the kernel's exp/ln mix never ping-pongs table loads (the default
    greedy per-function pick reloads ~10x per run, ~1.3us each)."""
    if os.environ.get("BASS_ACT_ROOT_JSON_PATH"):
        return
    try:
        import glob
        import json
        import tempfile

        import neuronxcc

        pwp = os.path.join(os.path.dirname(neuronxcc.__file__), "pwp",
                           "pwp_bin_trainium")
        info = json.load(open(os.path.join(pwp, "act_info.json")))
        keep_names = ["natural_log_exp_and_others",
                      "gelu_apprx_tanh_and_others"]
        keep = [s for s in info["act_func_sets"] if s["name"] in keep_names]
        keep.sort(key=lambda s: keep_names.index(s["name"]))
        if len(keep) != len(keep_names):
            return
        d = tempfile.mkdtemp(prefix="act_root_")
        for f in glob.glob(os.path.join(pwp, "*")):
            dst = os.path.join(d, os.path.basename(f))
            if not os.path.exists(dst):
                os.symlink(f, dst)
        out = dict(info)
        out["act_func_sets"] = keep
        patched = os.path.join(d, "act_info.json")
        os.unlink(patched)
        with open(patched, "w") as fh:
            json.dump(out, fh)
        # bacc pre-places the table loads itself (set id = index into
        # act_info.json) - patch its table lookup to match the trimmed json
        import concourse.hw_specs as hw_specs

        tables = {
            s["name"]: {AF.from_pwp(v) for v in s["act"].keys()} for s in keep
        }

        def _tables(arch, _t=tables):
            return _t

        hw_specs.get_activation_tables = _tables
        bacc.get_activation_tables = _tables
        os.environ["BASS_ACT_ROOT_JSON_PATH"] = patched
    except Exception:
        pass


def _heavy_pass(nc, psum, op_parts, rhs_tile, ncols, out_cb, rhs_cols=None):
    """out = Op @ X for a grouped rhs: Op given as list of k-packed lhsT
    sbuf tiles [128, 8, 1024] (bf16 hi [+ lo]); rhs_tile [128, 8, ncols]
    bf16. Calls out_cb(q, psum_tile[128, ncols]) per node tile q."""
    for q in range(NQ):
        ps = psum.tile([128, ncols], F32, tag="hv")
        n_mm = len(op_parts) * NQ
        i = 0
        for part in op_parts:
            for k in range(NQ):
                rhs = rhs_tile[:, k, :ncols] if rhs_cols is None else rhs_cols(k)
                nc.tensor.matmul(
                    ps[:],
                    lhsT=part[:, k, q * 128:(q + 1) * 128],
                    rhs=rhs,
                    start=(i == 0),
                    stop=(i == n_mm - 1),
                )
                i += 1
        out_cb(q, ps)


def build_bass():
    if "nc" in _BUILD_CACHE:
        return _BUILD_CACHE["nc"]
    _pin_act_table_set()
    nc = bacc.Bacc("TRN2", target_bir_lowering=False, debug=False,
                   num_devices=NCORES)
    dp = nc.declare_dram_parameter

    # --- kernel inputs (per-core host-prepped) ---
    # All small per-core constants ride ONE packed f32 param (the DMA
    # engines are descriptor-rate-bound at ~120ns/descriptor: each separate
    # [128,*] load costs 128 descriptors regardless of size).
    # packA cols: 0 btune | 1-2 rms1,rms2 | 3-19 bbc1 | 20-36 bbc2 |
    #   37-52 negA1 | 53-68 negA2 | 69-93 actbias | 94-101 ids(i32 bits) |
    #   102-229 identity
    PK_BT, PK_RMS, PK_BBC, PK_NEGA, PK_ACTB, PK_IDS, PK_ID = \
        0, 1, 3, 37, 69, 94, 102
    PKA_N = 230
    packA = dp("packA", [128, PKA_N], F32, isOutput=False)
    # packB cols (bf16): 0-16 wb1 | 17-33 wb2 | 34 ones
    packB = dp("packB", [128, 35], BF16, isOutput=False)
    lt_hi = dp("lt_hi", [128, NQ * 1024], BF16, isOutput=False)
    dt_hi = dp("dt_hi", [128, NQ * 1024], BF16, isOutput=False)
    xsT_a = dp("xsT_a", [128, 1024], BF16, isOutput=False)
    xsT_b = dp("xsT_b", [KD - 128, 1024], BF16, isOutput=False)
    wtune_a = dp("wtune_a", [128, 128], BF16, isOutput=False)
    wtune_b = dp("wtune_b", [KD - 128, 128], BF16, isOutput=False)
    mc = dp("mc", [NN, 2 * HS], F32, isOutput=False)  # [m1 hs | m2 hs]

    c1o = dp("c1o", [128, NQ, HS], F32, isOutput=True)
    c2o = dp("c2o", [128, NQ, HS], F32, isOutput=True)

    # collective bounce buffers (fp16 payload: halves collective bytes)
    F16 = mybir.dt.float16
    ag_in = nc.dram_tensor("ag_in", [HS, 1024], F16)
    ag_out = nc.dram_tensor("ag_out", [128, 1024], F16, addr_space="Shared")
    # dummy pre-warm collective target: absorbs the ~25us cross-core
    # rendezvous cost under stage-1 compute so the real AllGather is cheap
    dum_in = nc.dram_tensor("dum_in", [1, 128], F32)
    dum_out = nc.dram_tensor("dum_out", [NCORES, 128], F32, addr_space="Shared")

    with tile.TileContext(nc) as tc:
        with tc.tile_pool(name="const", bufs=1) as cst, \
             tc.tile_pool(name="work", bufs=1) as wk, \
             tc.tile_pool(name="psum", bufs=4, space="PSUM") as psum, \
             tc.tile_pool(name="psmall", bufs=2, space="PSUM") as psmall, \
             tc.tile_pool(name="ptrp", bufs=2, space="PSUM") as ptrp:

            # ---------- constant loads ----------
            pack_sb = cst.tile([128, PKA_N], F32, tag="packA")
            packb_sb = cst.tile([128, 35], BF16, tag="packB")
            xsT_a_sb = cst.tile([128, 1024], BF16, tag="xsTa")
            xsT_b_sb = cst.tile([KD - 128, 1024], BF16, tag="xsTb")
            wtune_a_sb = cst.tile([128, 128], BF16, tag="wta")
            wtune_b_sb = cst.tile([KD - 128, 128], BF16, tag="wtb")

            nc.sync.dma_start(out=pack_sb[:], in_=packA[:])

            # pre-warm the collective path: tiny dummy AllGather issued at
            # start so the cross-core rendezvous (~12us+) hides under
            # stage-1 compute and the real AllGather's algo starts promptly
            nc.sync.dma_start(out=dum_in[:], in_=pack_sb[0:1, 0:128])
            nc.gpsimd.collective_compute(
                "AllGather", OP.bypass,
                replica_groups=[list(range(NCORES))],
                ins=[dum_in[:]], outs=[dum_out[:]],
            )

            # memory-table gathers (early; both stages' rows in one pass
            # over the concatenated [NN, 32] table)
            mg_both = wk.tile([128, NQ, 2 * HS], F32, tag="mgb")
            for q in range(NQ):
                nc.gpsimd.indirect_dma_start(
                    out=mg_both[:, q, :],
                    out_offset=None,
                    in_=mc[:],
                    in_offset=bass.IndirectOffsetOnAxis(
                        ap=pack_sb[:, PK_IDS + q:PK_IDS + q + 1].bitcast(I32),
                        axis=0),
                )

            nc.sync.dma_start(out=packb_sb[:], in_=packB[:])
            nc.sync.dma_start(out=xsT_a_sb[:], in_=xsT_a[:])
            nc.sync.dma_start(out=xsT_b_sb[:], in_=xsT_b[:])
            nc.sync.dma_start(out=wtune_a_sb[:], in_=wtune_a[:])
            nc.sync.dma_start(out=wtune_b_sb[:], in_=wtune_b[:])

            # [128,1,*] views of packed consts for middle-dim broadcasts
            bbc_sb = [cst.tile([128, 1, HS + 1], F32, tag=f"bbc{s}", name=f"bbc_sb{s}") for s in range(2)]
            negA_t = [cst.tile([128, 1, HS], F32, tag=f"negA{s}", name=f"negA_t{s}") for s in range(2)]
            for s in range(2):
                nc.vector.tensor_copy(
                    out=bbc_sb[s][:, 0, :],
                    in_=pack_sb[:, PK_BBC + 17 * s:PK_BBC + 17 * (s + 1)])
                nc.vector.tensor_copy(
                    out=negA_t[s][:, 0, :],
                    in_=pack_sb[:, PK_NEGA + HS * s:PK_NEGA + HS * (s + 1)])
            wb_sb = [packb_sb[:, 17 * s:17 * (s + 1)] for s in range(2)]
            ones_ap = packb_sb[:, 34:35]
            ident_ap = pack_sb[:, PK_ID:PK_ID + 128]

            # operator loads (big; overlap with small pipeline)
            lt_sb = [cst.tile([128, NQ, 1024], BF16, tag="lt_hi", name="lt_hi_sb")]
            dt_sb = [cst.tile([128, NQ, 1024], BF16, tag="dt_hi", name="dt_hi_sb")]
            nc.sync.dma_start(out=lt_sb[0][:], in_=lt_hi[:])
            nc.sync.dma_start(out=dt_sb[0][:], in_=dt_hi[:])

            # zt^T = W_tune^T @ x_in^T + b_tune   [128 H, 1024 nodes] f32
            ztT = wk.tile([128, 1024], F32, tag="ztT")
            for hhalf in range(2):
                ps = psmall.tile([128, 512], F32, tag="sp")
                cols = slice(hhalf * 512, (hhalf + 1) * 512)
                nc.tensor.matmul(ps[:], lhsT=wtune_a_sb[:],
                                 rhs=xsT_a_sb[:, cols], start=True, stop=False)
                nc.tensor.matmul(ps[:], lhsT=wtune_b_sb[:],
                                 rhs=xsT_b_sb[:, cols], start=False, stop=True)
                nc.vector.tensor_scalar(out=ztT[:, cols], in0=ps[:],
                                        scalar1=pack_sb[:, PK_BT:PK_BT + 1],
                                        scalar2=None, op0=OP.add)

            c1T_full = wk.tile([128, 1024], mybir.dt.float16, tag="c1T_full")
            u2T = wk.tile([128, 1024], F32, tag="u2T")
            gtmp = wk.tile([128, 1024], F32, tag="gtmp")

            couts = (c1o, c2o)

            for s in range(2):  # the two SSM stages
                if s == 0:
                    base = ztT
                else:
                    # u2 = zt + gelu(c1) via the HW tanh-approx gelu table
                    nc.scalar.activation(gtmp[:], c1T_full[:],
                                         AF.Gelu_apprx_tanh)
                    nc.vector.tensor_tensor(out=u2T[:], in0=ztT[:],
                                            in1=gtmp[:], op=OP.add)
                    base = u2T

                # scaled bf16 lhsT for the B/delta matmuls
                baseS = wk.tile([128, 1024], BF16, tag=f"baseS{s}")
                nc.vector.tensor_scalar(out=baseS[:], in0=base[:],
                                        scalar1=pack_sb[:, PK_RMS + s:PK_RMS + s + 1],
                                        scalar2=None, op0=OP.mult)
                # squares (bf16) for the rms row-sums (DVE; keeps ACT on one
                # exp/ln table set)
                sq = wk.tile([128, 1024], BF16, tag=f"sq{s}")
                nc.vector.tensor_tensor(out=sq[:], in0=base[:], in1=base[:],
                                        op=OP.mult)

                # ss[p,q] = sum_H zt^2 ; rinv = 1/sqrt(ss/H) via exp/ln
                # (all 8 q-matmuls land in one PSUM tile; ACT reads PSUM)
                ssp = psmall.tile([128, NQ, 1], F32, tag="sp")
                for q in range(NQ):
                    nc.tensor.matmul(ssp[:, q, :],
                                     lhsT=sq[:, q * 128:(q + 1) * 128],
                                     rhs=ones_ap, start=True, stop=True)
                lnss = wk.tile([128, NQ, 1], F32, tag=f"lnss{s}")
                nc.scalar.activation(lnss[:], ssp[:], AF.Ln)
                rinv = wk.tile([128, NQ, 1], F32, tag=f"rinv{s}")
                nc.scalar.activation(rinv[:], lnss[:], AF.Exp, scale=-0.5,
                                     bias=pack_sb[:, PK_ACTB:PK_ACTB + 1])

                # B/delta matmuls + normalization fold (normal land, packed)
                psb = psmall.tile([128, NQ, HS + 1], F32, tag="sp")
                for q in range(NQ):
                    nc.tensor.matmul(psb[:, q, :],
                                     lhsT=baseS[:, q * 128:(q + 1) * 128],
                                     rhs=wb_sb[s], start=True, stop=True)
                BD = wk.tile([128, NQ, HS + 1], F32, tag=f"BD{s}")
                nc.vector.tensor_tensor(
                    out=BD[:], in0=psb[:],
                    in1=rinv[:].to_broadcast([128, NQ, HS + 1]), op=OP.mult)
                nc.vector.tensor_tensor(
                    out=BD[:], in0=BD[:],
                    in1=bbc_sb[s][:].to_broadcast([128, NQ, HS + 1]),
                    op=OP.add)

                # delta = softplus(BD[...,16]) = ln(1+exp(x)); the +1 rides
                # the Ln activation's bias port
                esp = wk.tile([128, NQ, 1], F32, tag=f"esp{s}")
                nc.scalar.activation(esp[:], BD[:, :, HS:HS + 1], AF.Exp)
                deltap = wk.tile([128, NQ, 1], F32, tag=f"deltap{s}")
                nc.scalar.activation(deltap[:], esp[:], AF.Ln, bias=1.0)

                # X = B*delta (bf16, straight into R0) ; dA = delta*negA ;
                # At = exp(dA) ; M = m_gather*At
                R0 = wk.tile([128, NQ, 2 * HS], BF16, tag=f"R0{s}")
                nc.vector.tensor_tensor(
                    out=R0[:, :, 0:HS], in0=BD[:, :, 0:HS],
                    in1=deltap[:].to_broadcast([128, NQ, HS]), op=OP.mult)
                dA = wk.tile([128, NQ, HS], F32, tag=f"dA{s}")
                nc.vector.tensor_tensor(
                    out=dA[:], in0=deltap[:].to_broadcast([128, NQ, HS]),
                    in1=negA_t[s][:].to_broadcast([128, NQ, HS]), op=OP.mult)
                At = wk.tile([128, NQ, HS], F32, tag=f"At{s}")
                nc.scalar.activation(At[:], dA[:], AF.Exp)
                Mf = wk.tile([128, NQ, HS], F32, tag=f"Mf{s}")
                nc.vector.tensor_tensor(
                    out=Mf[:], in0=mg_both[:, :, s * HS:(s + 1) * HS],
                    in1=At[:], op=OP.mult)
                nc.vector.tensor_copy(out=R0[:, :, HS:2 * HS], in_=Mf[:])

                # moments S_j[p,q,h] = sum_k w_k t_k^j exp(dA t_k). The
                # t_k^j factors ride the exp bias (ln(w_k t_k^j) columns of
                # actbias), so accumulation is pure adds on the otherwise
                # idle GpSimd engine. Chunks are interleaved between heavy
                # passes (see below) to fill ACT idle time without delaying
                # pass callbacks.
                Smom = [wk.tile([128, NQ, HS], BF16, tag=f"S{j}{s}",
                                name=f"S{j}{s}") for j in range(3)]

                def emit_moments(j, s=s, Smom=Smom, dA=dA):
                    wEs = []
                    for k in range(8):
                        wE = wk.tile([128, NQ, HS], BF16, tag=f"wE{s}_{j}_{k}",
                                     name=f"wE{s}_{j}_{k}")
                        nc.scalar.activation(
                            wE[:], dA[:], AF.Exp, scale=float(T_NODES[k]),
                            bias=pack_sb[:, PK_ACTB + 1 + 8 * j + k:
                                         PK_ACTB + 2 + 8 * j + k])
                        wEs.append(wE)
                    # pairwise tree add on GpSimd
                    for a, b in ((0, 1), (2, 3), (4, 5), (6, 7)):
                        nc.gpsimd.tensor_tensor(out=wEs[a][:], in0=wEs[a][:],
                                                in1=wEs[b][:], op=OP.add)
                    for a, b in ((0, 2), (4, 6)):
                        nc.gpsimd.tensor_tensor(out=wEs[a][:], in0=wEs[a][:],
                                                in1=wEs[b][:], op=OP.add)
                    nc.gpsimd.tensor_tensor(out=Smom[j][:], in0=wEs[0][:],
                                            in1=wEs[4][:], op=OP.add)

                # ---- heavy pass L1: L @ [X | M] -> LX, Y1 ----
                R1 = wk.tile([128, NQ, 3 * HS], BF16, tag=f"R1{s}")  # [V|M|Y1]
                nc.vector.tensor_copy(out=R1[:, :, HS:2 * HS],
                                      in_=R0[:, :, HS:2 * HS])

                def l1_cb(q, ps, s=s, R1=R1, R0=R0):
                    # V = X - REG*LX  (bf16 into R1) ; Y1 = psum[:,16:32]
                    nc.vector.scalar_tensor_tensor(
                        out=R1[:, q, 0:HS], in0=ps[:, 0:HS], scalar=-REG,
                        in1=R0[:, q, 0:HS], op0=OP.mult, op1=OP.add)
                    nc.scalar.activation(R1[:, q, 2 * HS:3 * HS],
                                         ps[:, HS:2 * HS], AF.Copy)

                _heavy_pass(nc, psum, lt_sb, R0, 2 * HS, l1_cb)

                # ---- heavy pass D1: D @ [V | M | Y1] -> U, UM, T1 ----
                R2 = wk.tile([128, NQ, 3 * HS], BF16, tag=f"R2{s}")  # [W1|U|UM]
                T1b = wk.tile([128, NQ, HS], BF16, tag=f"T1b{s}")

                def d1_cb(q, ps, R2=R2, T1b=T1b):
                    # split across DVE/ACT: ACT is the stage-2 bottleneck
                    nc.vector.tensor_copy(out=R2[:, q, HS:3 * HS],
                                          in_=ps[:, 0:2 * HS])
                    nc.scalar.activation(T1b[:, q, :], ps[:, 2 * HS:3 * HS],
                                         AF.Copy)

                _heavy_pass(nc, psum, dt_sb, R1, 3 * HS, d1_cb)
                emit_moments(0)

                # ---- heavy pass L2: L @ V -> W1 ----
                def l2_cb(q, ps, R2=R2):
                    nc.scalar.activation(R2[:, q, 0:HS], ps[:, 0:HS], AF.Copy)

                _heavy_pass(nc, psum, lt_sb[:1], R1, HS, l2_cb)
                emit_moments(1)

                # ---- heavy pass D2: D @ [W1 | U | UM] -> P, Q, T2 ----
                OUT2 = wk.tile([128, NQ, 3 * HS], BF16, tag=f"OUT2{s}")

                def d2_cb(q, ps, OUT2=OUT2):
                    nc.vector.tensor_copy(out=OUT2[:, q, :], in_=ps[:])

                _heavy_pass(nc, psum, dt_sb[:1], R2, 3 * HS, d2_cb)
                emit_moments(2)

                # ---- combine ----
                # S-products on GpSimd in parallel with the M-term chain on
                # DVE; DVE then folds everything.
                acc = wk.tile([128, NQ, HS], F32, tag=f"acc{s}")
                pV = wk.tile([128, NQ, HS], F32, tag=f"pV{s}")
                pU = wk.tile([128, NQ, HS], F32, tag=f"pU{s}")
                pP = wk.tile([128, NQ, HS], F32, tag=f"pP{s}")
                pQ = wk.tile([128, NQ, HS], F32, tag=f"pQ{s}")
                nc.gpsimd.tensor_tensor(out=pV[:], in0=R1[:, :, 0:HS],
                                        in1=Smom[0][:], op=OP.mult)
                nc.gpsimd.tensor_tensor(out=pU[:], in0=R2[:, :, HS:2 * HS],
                                        in1=Smom[1][:], op=OP.mult)
                nc.gpsimd.tensor_tensor(out=pP[:], in0=OUT2[:, :, 0:HS],
                                        in1=Smom[1][:], op=OP.mult)
                nc.gpsimd.tensor_tensor(out=pQ[:], in0=OUT2[:, :, HS:2 * HS],
                                        in1=Smom[2][:], op=OP.mult)
                # acc = M - REG*UM
                nc.vector.scalar_tensor_tensor(
                    out=acc[:], in0=R2[:, :, 2 * HS:3 * HS], scalar=-REG,
                    in1=Mf[:], op0=OP.mult, op1=OP.add)
                # + REG^2*T1
                nc.vector.scalar_tensor_tensor(
                    out=acc[:], in0=T1b[:], scalar=REG2, in1=acc[:],
                    op0=OP.mult, op1=OP.add)
                # + REG^2/2*T2
                nc.vector.scalar_tensor_tensor(
                    out=acc[:], in0=OUT2[:, :, 2 * HS:3 * HS], scalar=REG2 / 2,
                    in1=acc[:], op0=OP.mult, op1=OP.add)
                # + V*S0
                nc.vector.tensor_tensor(out=acc[:], in0=acc[:], in1=pV[:],
                                        op=OP.add)
                # - REG*U*S1
                nc.vector.scalar_tensor_tensor(
                    out=acc[:], in0=pU[:], scalar=-REG, in1=acc[:],
                    op0=OP.mult, op1=OP.add)
                # + REG^2*P*S1
                nc.vector.scalar_tensor_tensor(
                    out=acc[:], in0=pP[:], scalar=REG2, in1=acc[:],
                    op0=OP.mult, op1=OP.add)
                # + REG^2/2*Q*S2
                nc.vector.scalar_tensor_tensor(
                    out=acc[:], in0=pQ[:], scalar=REG2 / 2, in1=acc[:],
                    op0=OP.mult, op1=OP.add)

                # write output shard
                nc.sync.dma_start(out=couts[s][:], in_=acc[:])

                if s == 0:
                    # transpose c1 shard to [16,1024], AllGather to c1T_full
                    c1Ts = wk.tile([HS, 1024], mybir.dt.float16, tag="c1Ts")
                    for q in range(NQ):
                        pst = ptrp.tile([HS, 128], F32, tag="trp")
                        nc.tensor.transpose(pst[:], acc[:, q, :], ident_ap)
                        nc.vector.tensor_copy(
                            out=c1Ts[:, q * 128:(q + 1) * 128], in_=pst[:])
                    nc.sync.dma_start(out=ag_in[:], in_=c1Ts[:])
                    nc.gpsimd.collective_compute(
                        "AllGather", OP.bypass,
                        replica_groups=[list(range(NCORES))],
                        ins=[ag_in[:]], outs=[ag_out[:]],
                    )
                    nc.sync.dma_start(out=c1T_full[:], in_=ag_out[:])

    nc.compile()
    _BUILD_CACHE["nc"] = nc
    return nc


def _split_bf16(a):
    hi = a.astype(BF)
    lo = (a - hi.astype(np.float32)).astype(BF)
    return hi, lo


def _pack_kt(a_T):
    """[1024, 1024] (k-major rows) -> [128, 8*1024] partition-packed bf16 pair."""
    r = a_T.reshape(NQ, 128, 1024).transpose(1, 0, 2).reshape(128, NQ * 1024)
    return r


def kernel(**inputs):
    out, _ = _run(inputs, trace=False)
    return out


def _run(inputs, trace=False, trace_kwargs=None):
    inp = {k: np.asarray(v) for k, v in inputs.items()}
    L = inp["L_agg"].astype(np.float32)
    D = inp["delta_L_agg"].astype(np.float32)
    x_sub = inp["x_sub"].astype(np.float32)
    m1 = inp["m1_vec"].astype(np.float32)
    m2 = inp["m2_vec"].astype(np.float32)
    names = inp["names_table"].astype(np.float32)
    rms1 = inp["rms1_scale"].astype(np.float32)
    rms2 = inp["rms2_scale"].astype(np.float32)
    W_tune = inp["W_tune"].astype(np.float32)
    b_tune = inp["b_tune"].astype(np.float32)
    W_B1 = inp["W_B1"].astype(np.float32)
    b_B1 = inp["b_B1"].astype(np.float32)
    W_B2 = inp["W_B2"].astype(np.float32)
    b_B2 = inp["b_B2"].astype(np.float32)
    W_dt = inp["W_dt"].astype(np.float32)
    b_dt = inp["b_dt"].astype(np.float32)
    A1 = inp["A_log_1"].astype(np.float32)
    A2 = inp["A_log_2"].astype(np.float32)
    tsrc = np.asarray(inp["target_src"]).astype(np.int64)
    tdst = np.asarray(inp["target_dst"]).astype(np.int64)
    aids = np.asarray(inp["active_input_ids"]).astype(np.int64)

    # x_in = [x_sub | neigh]; the names_table neighbor embedding (ED=1)
    neigh = np.zeros((NA, 2 * ED), np.float32)
    neigh[:E, :ED] = names[tsrc]
    neigh[:E, ED:] = names[tdst]
    neigh[E:2 * E, :ED] = names[tdst]
    neigh[E:2 * E, ED:] = names[tsrc]
    x_in = np.concatenate([x_sub, neigh], axis=1)  # [1024, 174]
    xsT = np.ascontiguousarray(x_in.T)  # [174, 1024]

    lt_hi = _pack_kt(np.ascontiguousarray(L.T).astype(BF))
    dt_hi = _pack_kt(np.ascontiguousarray(D.T).astype(BF))

    ids_p = np.ascontiguousarray(
        aids.astype(np.int32).reshape(NQ, 128).T)  # [128p, 8q]

    negA1_full = -np.exp(A1)  # [128]
    negA2_full = -np.exp(A2)

    common = {
        "lt_hi": lt_hi, "dt_hi": dt_hi,
        "xsT_a": xsT[:128].astype(BF),
        "xsT_b": np.ascontiguousarray(xsT[128:]).astype(BF),
        "wtune_a": W_tune[:128].astype(BF),
        "wtune_b": np.ascontiguousarray(W_tune[128:]).astype(BF),
    }
    actb = np.array(
        [0.5 * np.log(H)]
        + [np.log(w) for w in T_W]
        + [np.log(w * t) for w, t in zip(T_W, T_NODES)]
        + [np.log(w * t * t) for w, t in zip(T_W, T_NODES)],
        np.float32)  # [25]

    in_maps = []
    for c in range(NCORES):
        hs = slice(c * HS, (c + 1) * HS)
        wb1c = np.concatenate([W_B1[:, hs], W_dt], axis=1).astype(BF)
        wb2c = np.concatenate([W_B2[:, hs], W_dt], axis=1).astype(BF)
        # packA: 0 btune | 1-2 rms | 3-36 bbc1,bbc2 | 37-68 negA1,negA2 |
        # 69-93 actb | 94-101 ids bits | 102-229 identity
        packa = np.zeros((128, 230), np.float32)
        packa[:, 0] = b_tune
        packa[:, 1] = rms1
        packa[:, 2] = rms2
        packa[:, 3:20] = np.concatenate([b_B1[hs], b_dt])
        packa[:, 20:37] = np.concatenate([b_B2[hs], b_dt])
        packa[:, 37:53] = negA1_full[hs]
        packa[:, 53:69] = negA2_full[hs]
        packa[:, 69:94] = actb
        packa[:, 94:102] = ids_p.view(np.float32)
        packa[:, 102:230] = np.eye(128, dtype=np.float32)
        packb = np.concatenate(
            [wb1c, wb2c, np.ones((128, 1), BF)], axis=1)
        in_maps.append({
            **common,
            "packA": packa, "packB": np.ascontiguousarray(packb),
            "mc": np.ascontiguousarray(
                np.concatenate([m1[:, hs], m2[:, hs]], axis=1)),
        })

    nc = build_bass()
    res = run_bass_kernel_spmd(nc, in_maps, core_ids=list(range(NCORES)),
                               trace=trace, **(trace_kwargs or {}))

    out = np.zeros((2, NA, H), np.float32)
    for c in range(NCORES):
        hs = slice(c * HS, (c + 1) * HS)
        # packed [128p, 8q, 16h] -> [1024, 16]
        out[0][:, hs] = res.results[c]["c1o"].transpose(1, 0, 2).reshape(NA, HS)
        out[1][:, hs] = res.results[c]["c2o"].transpose(1, 0, 2).reshape(NA, HS)
    return out, res



# revision 65
# speedup vs baseline: 1.0670x; 1.0670x over previous
"""Trainium2 Bass kernel for nn_MemoryModel (scatter_memory, 8 cores).

Math (per stage): the 8-point Gauss-Legendre quadrature over matrix
polynomials collapses algebraically:

  LHS_k = I - REG*t_k*D + REG^2*(t_k*D@L + t_k^2/2*D@D)      (D=delta_L, L=L_agg)
  integral = sum_k (LHS_k @ (w_k*V)) * exp(dA*t_k)
           = V*S0 - REG*U*S1 + REG^2*P*S1 + REG^2/2*Q*S2
  with V = X - REG*(L@X),  U = D@V, W1 = L@V, P = D@W1, Q = D@U
  and moments S_j = sum_k w_k t_k^j exp(dA t_k)   (elementwise [n,H])
  As_bar @ M = M - REG*(D@M) + REG^2*(D@(L@M)) + REG^2/2*(D@(D@M))

So each stage costs 9 matmuls of [1024,1024]@[1024,16] per core instead of
nine n^3 products; no n^3 matmul anywhere.

Sharding: H=128 is column-sharded 8 ways (16 cols/core). The [1024,1024]
operators (as transposed, k-tile-packed bf16 hi/lo splits) are replicated;
the per-node small pipeline runs in "transposed land" (H on partitions)
replicated on every core; heavy chains run per-core on the 16-column shard
in node-packed layout [128p, 8q, 16h] (node = 128q+p). The memory tables
m1/m2 are column-sharded [100000,16] per core and gathered on-device with
indirect DMA. One AllGather ([16,1024] -> [128,1024]) carries stage-1
output c1^T to all cores for stage 2.
"""
import os
import sys

import numpy as np

for _p in ("/opt/trn_rl_repo", "/root/.axon_site/_ro/trn_rl_repo"):
    if os.path.isdir(_p) and _p not in sys.path:
        sys.path.insert(0, _p)

import ml_dtypes  # noqa: E402
import concourse.bass as bass  # noqa: E402
import concourse.bacc as bacc  # noqa: E402
import concourse.mybir as mybir  # noqa: E402
import concourse.tile as tile  # noqa: E402
from concourse.bass_utils import run_bass_kernel_spmd  # noqa: E402

F32 = mybir.dt.float32
BF16 = mybir.dt.bfloat16
I32 = mybir.dt.int32
AF = mybir.ActivationFunctionType
OP = mybir.AluOpType
BF = ml_dtypes.bfloat16

NA, H, DIN, E, NN, ED = 1024, 128, 172, 256, 100000, 1
KD = DIN + 2 * ED  # 174
REG = 0.1
REG2 = REG * REG
NCORES = 8
HS = 16  # H columns per core
NQ = 8  # node tiles (1024/128)

_gl_nodes = [-0.1834346424956498, -0.525532409916329, -0.7966664774136267,
             -0.9602898564975363, 0.1834346424956498, 0.525532409916329,
             0.7966664774136267, 0.9602898564975363]
_gl_w = [0.362683783378362, 0.3137066458778873, 0.2223810344533745,
         0.1012285362903763] * 2
T_NODES = [0.5 * (x + 1.0) for x in _gl_nodes]
T_W = [0.5 * w for w in _gl_w]

SPLIT_FIRST = False  # hi/lo bf16 split for the first-order passes (L1, D1)

_BUILD_CACHE = {}


def _pin_act_table_set():
    """Restrict walrus's ACT-table choice to natural_log_exp_and_others so
    the kernel's exp/ln mix never ping-pongs table loads (the default
    greedy per-function pick reloads ~10x per run, ~1.3us each)."""
    if os.environ.get("BASS_ACT_ROOT_JSON_PATH"):
        return
    try:
        import glob
        import json
        import tempfile

        import neuronxcc

        pwp = os.path.join(os.path.dirname(neuronxcc.__file__), "pwp",
                           "pwp_bin_trainium")
        info = json.load(open(os.path.join(pwp, "act_info.json")))
        keep_names = ["natural_log_exp_and_others",
                      "gelu_apprx_tanh_and_others"]
        keep = [s for s in info["act_func_sets"] if s["name"] in keep_names]
        keep.sort(key=lambda s: keep_names.index(s["name"]))
        if len(keep) != len(keep_names):
            return
        d = tempfile.mkdtemp(prefix="act_root_")
        for f in glob.glob(os.path.join(pwp, "*")):
            dst = os.path.join(d, os.path.basename(f))
            if not os.path.exists(dst):
                os.symlink(f, dst)
        out = dict(info)
        out["act_func_sets"] = keep
        patched = os.path.join(d, "act_info.json")
        os.unlink(patched)
        with open(patched, "w") as fh:
            json.dump(out, fh)
        # bacc pre-places the table loads itself (set id = index into
        # act_info.json) - patch its table lookup to match the trimmed json
        import concourse.hw_specs as hw_specs

        tables = {
            s["name"]: {AF.from_pwp(v) for v in s["act"].keys()} for s in keep
        }

        def _tables(arch, _t=tables):
            return _t

        hw_specs.get_activation_tables = _tables
        bacc.get_activation_tables = _tables
        os.environ["BASS_ACT_ROOT_JSON_PATH"] = patched
    except Exception:
        pass


def _heavy_pass(nc, psum, op_parts, rhs_tile, ncols, out_cb, rhs_cols=None):
    """out = Op @ X for a grouped rhs: Op given as list of k-packed lhsT
    sbuf tiles [128, 8, 1024] (bf16 hi [+ lo]); rhs_tile [128, 8, ncols]
    bf16. Calls out_cb(q, psum_tile[128, ncols]) per node tile q."""
    for q in range(NQ):
        ps = psum.tile([128, ncols], F32, tag="hv")
        n_mm = len(op_parts) * NQ
        i = 0
        for part in op_parts:
            for k in range(NQ):
                rhs = rhs_tile[:, k, :ncols] if rhs_cols is None else rhs_cols(k)
                nc.tensor.matmul(
                    ps[:],
                    lhsT=part[:, k, q * 128:(q + 1) * 128],
                    rhs=rhs,
                    start=(i == 0),
                    stop=(i == n_mm - 1),
                )
                i += 1
        out_cb(q, ps)


def build_bass():
    if "nc" in _BUILD_CACHE:
        return _BUILD_CACHE["nc"]
    _pin_act_table_set()
    nc = bacc.Bacc("TRN2", target_bir_lowering=False, debug=False,
                   num_devices=NCORES)
    dp = nc.declare_dram_parameter

    # --- kernel inputs (per-core host-prepped) ---
    # All small per-core constants ride ONE packed f32 param (the DMA
    # engines are descriptor-rate-bound at ~120ns/descriptor: each separate
    # [128,*] load costs 128 descriptors regardless of size).
    # packA cols: 0 btune | 1-2 rms1,rms2 | 3-19 bbc1 | 20-36 bbc2 |
    #   37-52 negA1 | 53-68 negA2 | 69-93 actbias | 94-101 ids(i32 bits) |
    #   102-229 identity
    PK_BT, PK_RMS, PK_BBC, PK_NEGA, PK_ACTB, PK_IDS, PK_ID = \
        0, 1, 3, 37, 69, 94, 102
    PKA_N = 230
    packA = dp("packA", [128, PKA_N], F32, isOutput=False)
    # packB cols (bf16): 0-16 wb1 | 17-33 wb2 | 34 ones
    packB = dp("packB", [128, 35], BF16, isOutput=False)
    lt_hi = dp("lt_hi", [128, NQ * 1024], BF16, isOutput=False)
    dt_hi = dp("dt_hi", [128, NQ * 1024], BF16, isOutput=False)
    xsT_a = dp("xsT_a", [128, 1024], BF16, isOutput=False)
    xsT_b = dp("xsT_b", [KD - 128, 1024], BF16, isOutput=False)
    wtune_a = dp("wtune_a", [128, 128], BF16, isOutput=False)
    wtune_b = dp("wtune_b", [KD - 128, 128], BF16, isOutput=False)
    mc = dp("mc", [NN, 2 * HS], F32, isOutput=False)  # [m1 hs | m2 hs]

    c1o = dp("c1o", [128, NQ, HS], F32, isOutput=True)
    c2o = dp("c2o", [128, NQ, HS], F32, isOutput=True)

    # collective bounce buffers (fp16 payload: halves collective bytes)
    F16 = mybir.dt.float16
    ag_in = nc.dram_tensor("ag_in", [HS, 1024], F16)
    ag_out = nc.dram_tensor("ag_out", [128, 1024], F16, addr_space="Shared")
    # dummy pre-warm collective target: absorbs the ~25us cross-core
    # rendezvous cost under stage-1 compute so the real AllGather is cheap
    dum_in = nc.dram_tensor("dum_in", [1, 128], F32)
    dum_out = nc.dram_tensor("dum_out", [NCORES, 128], F32, addr_space="Shared")

    with tile.TileContext(nc) as tc:
        with tc.tile_pool(name="const", bufs=1) as cst, \
             tc.tile_pool(name="work", bufs=1) as wk, \
             tc.tile_pool(name="psum", bufs=4, space="PSUM") as psum, \
             tc.tile_pool(name="psmall", bufs=2, space="PSUM") as psmall, \
             tc.tile_pool(name="ptrp", bufs=2, space="PSUM") as ptrp:

            # ---------- constant loads ----------
            pack_sb = cst.tile([128, PKA_N], F32, tag="packA")
            packb_sb = cst.tile([128, 35], BF16, tag="packB")
            xsT_a_sb = cst.tile([128, 1024], BF16, tag="xsTa")
            xsT_b_sb = cst.tile([KD - 128, 1024], BF16, tag="xsTb")
            wtune_a_sb = cst.tile([128, 128], BF16, tag="wta")
            wtune_b_sb = cst.tile([KD - 128, 128], BF16, tag="wtb")

            nc.sync.dma_start(out=pack_sb[:], in_=packA[:])

            # pre-warm the collective path: tiny dummy AllGather issued at
            # start so the cross-core rendezvous (~12us+) hides under
            # stage-1 compute and the real AllGather's algo starts promptly.
            # dram->dram feed: no SBUF dependency, fires as the first DMA.
            nc.sync.dma_start(out=dum_in[:], in_=packA[0:1, 0:128])
            nc.gpsimd.collective_compute(
                "AllGather", OP.bypass,
                replica_groups=[list(range(NCORES))],
                ins=[dum_in[:]], outs=[dum_out[:]],
            )

            # memory-table gathers (early; both stages' rows in one pass
            # over the concatenated [NN, 32] table)
            mg_both = wk.tile([128, NQ, 2 * HS], F32, tag="mgb")
            for q in range(NQ):
                nc.gpsimd.indirect_dma_start(
                    out=mg_both[:, q, :],
                    out_offset=None,
                    in_=mc[:],
                    in_offset=bass.IndirectOffsetOnAxis(
                        ap=pack_sb[:, PK_IDS + q:PK_IDS + q + 1].bitcast(I32),
                        axis=0),
                )

            nc.sync.dma_start(out=packb_sb[:], in_=packB[:])
            nc.sync.dma_start(out=xsT_a_sb[:], in_=xsT_a[:])
            nc.sync.dma_start(out=xsT_b_sb[:], in_=xsT_b[:])
            nc.sync.dma_start(out=wtune_a_sb[:], in_=wtune_a[:])
            nc.sync.dma_start(out=wtune_b_sb[:], in_=wtune_b[:])

            # [128,1,*] views of packed consts for middle-dim broadcasts
            bbc_sb = [cst.tile([128, 1, HS + 1], F32, tag=f"bbc{s}", name=f"bbc_sb{s}") for s in range(2)]
            negA_t = [cst.tile([128, 1, HS], F32, tag=f"negA{s}", name=f"negA_t{s}") for s in range(2)]
            for s in range(2):
                nc.vector.tensor_copy(
                    out=bbc_sb[s][:, 0, :],
                    in_=pack_sb[:, PK_BBC + 17 * s:PK_BBC + 17 * (s + 1)])
                nc.vector.tensor_copy(
                    out=negA_t[s][:, 0, :],
                    in_=pack_sb[:, PK_NEGA + HS * s:PK_NEGA + HS * (s + 1)])
            wb_sb = [packb_sb[:, 17 * s:17 * (s + 1)] for s in range(2)]
            ones_ap = packb_sb[:, 34:35]
            ident_ap = pack_sb[:, PK_ID:PK_ID + 128]

            # operator loads (big; overlap with small pipeline)
            lt_sb = [cst.tile([128, NQ, 1024], BF16, tag="lt_hi", name="lt_hi_sb")]
            dt_sb = [cst.tile([128, NQ, 1024], BF16, tag="dt_hi", name="dt_hi_sb")]
            nc.sync.dma_start(out=lt_sb[0][:], in_=lt_hi[:])
            nc.sync.dma_start(out=dt_sb[0][:], in_=dt_hi[:])

            # zt^T = W_tune^T @ x_in^T + b_tune   [128 H, 1024 nodes] f32
            ztT = wk.tile([128, 1024], F32, tag="ztT")
            for hhalf in range(2):
                ps = psmall.tile([128, 512], F32, tag="sp")
                cols = slice(hhalf * 512, (hhalf + 1) * 512)
                nc.tensor.matmul(ps[:], lhsT=wtune_a_sb[:],
                                 rhs=xsT_a_sb[:, cols], start=True, stop=False)
                nc.tensor.matmul(ps[:], lhsT=wtune_b_sb[:],
                                 rhs=xsT_b_sb[:, cols], start=False, stop=True)
                nc.vector.tensor_scalar(out=ztT[:, cols], in0=ps[:],
                                        scalar1=pack_sb[:, PK_BT:PK_BT + 1],
                                        scalar2=None, op0=OP.add)

            c1T_full = wk.tile([128, 1024], mybir.dt.float16, tag="c1T_full")
            u2T = wk.tile([128, 1024], F32, tag="u2T")
            gtmp = wk.tile([128, 1024], F32, tag="gtmp")

            couts = (c1o, c2o)

            for s in range(2):  # the two SSM stages
                if s == 0:
                    base = ztT
                else:
                    # u2 = zt + gelu(c1) via the HW tanh-approx gelu table
                    nc.scalar.activation(gtmp[:], c1T_full[:],
                                         AF.Gelu_apprx_tanh)
                    nc.vector.tensor_tensor(out=u2T[:], in0=ztT[:],
                                            in1=gtmp[:], op=OP.add)
                    base = u2T

                # scaled bf16 lhsT for the B/delta matmuls
                baseS = wk.tile([128, 1024], BF16, tag=f"baseS{s}")
                nc.vector.tensor_scalar(out=baseS[:], in0=base[:],
                                        scalar1=pack_sb[:, PK_RMS + s:PK_RMS + s + 1],
                                        scalar2=None, op0=OP.mult)
                # squares (bf16) for the rms row-sums (DVE; keeps ACT on one
                # exp/ln table set)
                sq = wk.tile([128, 1024], BF16, tag=f"sq{s}")
                nc.vector.tensor_tensor(out=sq[:], in0=base[:], in1=base[:],
                                        op=OP.mult)

                # ss[p,q] = sum_H zt^2 ; rinv = 1/sqrt(ss/H) via exp/ln
                # (all 8 q-matmuls land in one PSUM tile; ACT reads PSUM)
                ssp = psmall.tile([128, NQ, 1], F32, tag="sp")
                for q in range(NQ):
                    nc.tensor.matmul(ssp[:, q, :],
                                     lhsT=sq[:, q * 128:(q + 1) * 128],
                                     rhs=ones_ap, start=True, stop=True)
                lnss = wk.tile([128, NQ, 1], F32, tag=f"lnss{s}")
                nc.scalar.activation(lnss[:], ssp[:], AF.Ln)
                rinv = wk.tile([128, NQ, 1], F32, tag=f"rinv{s}")
                nc.scalar.activation(rinv[:], lnss[:], AF.Exp, scale=-0.5,
                                     bias=pack_sb[:, PK_ACTB:PK_ACTB + 1])

                # B/delta matmuls + normalization fold (normal land, packed)
                psb = psmall.tile([128, NQ, HS + 1], F32, tag="sp")
                for q in range(NQ):
                    nc.tensor.matmul(psb[:, q, :],
                                     lhsT=baseS[:, q * 128:(q + 1) * 128],
                                     rhs=wb_sb[s], start=True, stop=True)
                BD = wk.tile([128, NQ, HS + 1], F32, tag=f"BD{s}")
                nc.vector.tensor_tensor(
                    out=BD[:], in0=psb[:],
                    in1=rinv[:].to_broadcast([128, NQ, HS + 1]), op=OP.mult)
                nc.vector.tensor_tensor(
                    out=BD[:], in0=BD[:],
                    in1=bbc_sb[s][:].to_broadcast([128, NQ, HS + 1]),
                    op=OP.add)

                # delta = softplus(BD[...,16]) = ln(1+exp(x)); the +1 rides
                # the Ln activation's bias port
                esp = wk.tile([128, NQ, 1], F32, tag=f"esp{s}")
                nc.scalar.activation(esp[:], BD[:, :, HS:HS + 1], AF.Exp)
                deltap = wk.tile([128, NQ, 1], F32, tag=f"deltap{s}")
                nc.scalar.activation(deltap[:], esp[:], AF.Ln, bias=1.0)

                # X = B*delta (bf16, straight into R0) ; dA = delta*negA ;
                # At = exp(dA) ; M = m_gather*At
                R0 = wk.tile([128, NQ, 2 * HS], BF16, tag=f"R0{s}")
                nc.vector.tensor_tensor(
                    out=R0[:, :, 0:HS], in0=BD[:, :, 0:HS],
                    in1=deltap[:].to_broadcast([128, NQ, HS]), op=OP.mult)
                dA = wk.tile([128, NQ, HS], F32, tag=f"dA{s}")
                nc.vector.tensor_tensor(
                    out=dA[:], in0=deltap[:].to_broadcast([128, NQ, HS]),
                    in1=negA_t[s][:].to_broadcast([128, NQ, HS]), op=OP.mult)
                At = wk.tile([128, NQ, HS], F32, tag=f"At{s}")
                nc.scalar.activation(At[:], dA[:], AF.Exp)
                Mf = wk.tile([128, NQ, HS], F32, tag=f"Mf{s}")
                nc.vector.tensor_tensor(
                    out=Mf[:], in0=mg_both[:, :, s * HS:(s + 1) * HS],
                    in1=At[:], op=OP.mult)
                nc.vector.tensor_copy(out=R0[:, :, HS:2 * HS], in_=Mf[:])

                # moments S_j[p,q,h] = sum_k w_k t_k^j exp(dA t_k). The
                # t_k^j factors ride the exp bias (ln(w_k t_k^j) columns of
                # actbias), so accumulation is pure adds on the otherwise
                # idle GpSimd engine. Chunks are interleaved between heavy
                # passes (see below) to fill ACT idle time without delaying
                # pass callbacks.
                Smom = [wk.tile([128, NQ, HS], BF16, tag=f"S{j}{s}",
                                name=f"S{j}{s}") for j in range(3)]

                def emit_moments(j, s=s, Smom=Smom, dA=dA):
                    wEs = []
                    for k in range(8):
                        wE = wk.tile([128, NQ, HS], BF16, tag=f"wE{s}_{j}_{k}",
                                     name=f"wE{s}_{j}_{k}")
                        nc.scalar.activation(
                            wE[:], dA[:], AF.Exp, scale=float(T_NODES[k]),
                            bias=pack_sb[:, PK_ACTB + 1 + 8 * j + k:
                                         PK_ACTB + 2 + 8 * j + k])
                        wEs.append(wE)
                    # pairwise tree add on GpSimd
                    for a, b in ((0, 1), (2, 3), (4, 5), (6, 7)):
                        nc.gpsimd.tensor_tensor(out=wEs[a][:], in0=wEs[a][:],
                                                in1=wEs[b][:], op=OP.add)
                    for a, b in ((0, 2), (4, 6)):
                        nc.gpsimd.tensor_tensor(out=wEs[a][:], in0=wEs[a][:],
                                                in1=wEs[b][:], op=OP.add)
                    nc.gpsimd.tensor_tensor(out=Smom[j][:], in0=wEs[0][:],
                                            in1=wEs[4][:], op=OP.add)

                # ---- heavy pass L1: L @ [X | M] -> LX, Y1 ----
                R1 = wk.tile([128, NQ, 3 * HS], BF16, tag=f"R1{s}")  # [V|M|Y1]
                nc.vector.tensor_copy(out=R1[:, :, HS:2 * HS],
                                      in_=R0[:, :, HS:2 * HS])

                def l1_cb(q, ps, s=s, R1=R1, R0=R0):
                    # V = X - REG*LX  (bf16 into R1) ; Y1 = psum[:,16:32]
                    nc.vector.scalar_tensor_tensor(
                        out=R1[:, q, 0:HS], in0=ps[:, 0:HS], scalar=-REG,
                        in1=R0[:, q, 0:HS], op0=OP.mult, op1=OP.add)
                    nc.scalar.activation(R1[:, q, 2 * HS:3 * HS],
                                         ps[:, HS:2 * HS], AF.Copy)

                _heavy_pass(nc, psum, lt_sb, R0, 2 * HS, l1_cb)

                # ---- heavy pass D1: D @ [V | M | Y1] -> U, UM, T1 ----
                R2 = wk.tile([128, NQ, 3 * HS], BF16, tag=f"R2{s}")  # [W1|U|UM]
                T1b = wk.tile([128, NQ, HS], BF16, tag=f"T1b{s}")

                def d1_cb(q, ps, R2=R2, T1b=T1b):
                    # split across DVE/ACT: ACT is the stage-2 bottleneck
                    nc.vector.tensor_copy(out=R2[:, q, HS:3 * HS],
                                          in_=ps[:, 0:2 * HS])
                    nc.scalar.activation(T1b[:, q, :], ps[:, 2 * HS:3 * HS],
                                         AF.Copy)

                _heavy_pass(nc, psum, dt_sb, R1, 3 * HS, d1_cb)
                emit_moments(0)

                # ---- heavy pass L2: L @ V -> W1 ----
                def l2_cb(q, ps, R2=R2):
                    nc.scalar.activation(R2[:, q, 0:HS], ps[:, 0:HS], AF.Copy)

                _heavy_pass(nc, psum, lt_sb[:1], R1, HS, l2_cb)
                emit_moments(1)

                # ---- heavy pass D2: D @ [W1 | U | UM] -> P, Q, T2 ----
                OUT2 = wk.tile([128, NQ, 3 * HS], BF16, tag=f"OUT2{s}")

                def d2_cb(q, ps, OUT2=OUT2):
                    nc.vector.tensor_copy(out=OUT2[:, q, :], in_=ps[:])

                _heavy_pass(nc, psum, dt_sb[:1], R2, 3 * HS, d2_cb)
                emit_moments(2)

                # ---- combine ----
                # S-products on GpSimd in parallel with the M-term chain on
                # DVE; DVE then folds everything.
                acc = wk.tile([128, NQ, HS], F32, tag=f"acc{s}")
                pV = wk.tile([128, NQ, HS], F32, tag=f"pV{s}")
                pU = wk.tile([128, NQ, HS], F32, tag=f"pU{s}")
                pP = wk.tile([128, NQ, HS], F32, tag=f"pP{s}")
                pQ = wk.tile([128, NQ, HS], F32, tag=f"pQ{s}")
                nc.gpsimd.tensor_tensor(out=pV[:], in0=R1[:, :, 0:HS],
                                        in1=Smom[0][:], op=OP.mult)
                nc.gpsimd.tensor_tensor(out=pU[:], in0=R2[:, :, HS:2 * HS],
                                        in1=Smom[1][:], op=OP.mult)
                nc.gpsimd.tensor_tensor(out=pP[:], in0=OUT2[:, :, 0:HS],
                                        in1=Smom[1][:], op=OP.mult)
                nc.gpsimd.tensor_tensor(out=pQ[:], in0=OUT2[:, :, HS:2 * HS],
                                        in1=Smom[2][:], op=OP.mult)
                # acc = M - REG*UM
                nc.vector.scalar_tensor_tensor(
                    out=acc[:], in0=R2[:, :, 2 * HS:3 * HS], scalar=-REG,
                    in1=Mf[:], op0=OP.mult, op1=OP.add)
                # + REG^2*T1
                nc.vector.scalar_tensor_tensor(
                    out=acc[:], in0=T1b[:], scalar=REG2, in1=acc[:],
                    op0=OP.mult, op1=OP.add)
                # + REG^2/2*T2
                nc.vector.scalar_tensor_tensor(
                    out=acc[:], in0=OUT2[:, :, 2 * HS:3 * HS], scalar=REG2 / 2,
                    in1=acc[:], op0=OP.mult, op1=OP.add)
                # + V*S0
                nc.vector.tensor_tensor(out=acc[:], in0=acc[:], in1=pV[:],
                                        op=OP.add)
                # - REG*U*S1
                nc.vector.scalar_tensor_tensor(
                    out=acc[:], in0=pU[:], scalar=-REG, in1=acc[:],
                    op0=OP.mult, op1=OP.add)
                # + REG^2*P*S1
                nc.vector.scalar_tensor_tensor(
                    out=acc[:], in0=pP[:], scalar=REG2, in1=acc[:],
                    op0=OP.mult, op1=OP.add)
                # + REG^2/2*Q*S2
                nc.vector.scalar_tensor_tensor(
                    out=acc[:], in0=pQ[:], scalar=REG2 / 2, in1=acc[:],
                    op0=OP.mult, op1=OP.add)

                # write output shard
                nc.sync.dma_start(out=couts[s][:], in_=acc[:])

                if s == 0:
                    # transpose c1 shard to [16,1024], AllGather to c1T_full
                    c1Ts = wk.tile([HS, 1024], mybir.dt.float16, tag="c1Ts")
                    for q in range(NQ):
                        pst = ptrp.tile([HS, 128], F32, tag="trp")
                        nc.tensor.transpose(pst[:], acc[:, q, :], ident_ap)
                        nc.vector.tensor_copy(
                            out=c1Ts[:, q * 128:(q + 1) * 128], in_=pst[:])
                    nc.sync.dma_start(out=ag_in[:], in_=c1Ts[:])
                    nc.gpsimd.collective_compute(
                        "AllGather", OP.bypass,
                        replica_groups=[list(range(NCORES))],
                        ins=[ag_in[:]], outs=[ag_out[:]],
                    )
                    nc.sync.dma_start(out=c1T_full[:], in_=ag_out[:])

    nc.compile()
    _BUILD_CACHE["nc"] = nc
    return nc


def _split_bf16(a):
    hi = a.astype(BF)
    lo = (a - hi.astype(np.float32)).astype(BF)
    return hi, lo


def _pack_kt(a_T):
    """[1024, 1024] (k-major rows) -> [128, 8*1024] partition-packed bf16 pair."""
    r = a_T.reshape(NQ, 128, 1024).transpose(1, 0, 2).reshape(128, NQ * 1024)
    return r


def kernel(**inputs):
    out, _ = _run(inputs, trace=False)
    return out


def _run(inputs, trace=False, trace_kwargs=None):
    inp = {k: np.asarray(v) for k, v in inputs.items()}
    L = inp["L_agg"].astype(np.float32)
    D = inp["delta_L_agg"].astype(np.float32)
    x_sub = inp["x_sub"].astype(np.float32)
    m1 = inp["m1_vec"].astype(np.float32)
    m2 = inp["m2_vec"].astype(np.float32)
    names = inp["names_table"].astype(np.float32)
    rms1 = inp["rms1_scale"].astype(np.float32)
    rms2 = inp["rms2_scale"].astype(np.float32)
    W_tune = inp["W_tune"].astype(np.float32)
    b_tune = inp["b_tune"].astype(np.float32)
    W_B1 = inp["W_B1"].astype(np.float32)
    b_B1 = inp["b_B1"].astype(np.float32)
    W_B2 = inp["W_B2"].astype(np.float32)
    b_B2 = inp["b_B2"].astype(np.float32)
    W_dt = inp["W_dt"].astype(np.float32)
    b_dt = inp["b_dt"].astype(np.float32)
    A1 = inp["A_log_1"].astype(np.float32)
    A2 = inp["A_log_2"].astype(np.float32)
    tsrc = np.asarray(inp["target_src"]).astype(np.int64)
    tdst = np.asarray(inp["target_dst"]).astype(np.int64)
    aids = np.asarray(inp["active_input_ids"]).astype(np.int64)

    # x_in = [x_sub | neigh]; the names_table neighbor embedding (ED=1)
    neigh = np.zeros((NA, 2 * ED), np.float32)
    neigh[:E, :ED] = names[tsrc]
    neigh[:E, ED:] = names[tdst]
    neigh[E:2 * E, :ED] = names[tdst]
    neigh[E:2 * E, ED:] = names[tsrc]
    x_in = np.concatenate([x_sub, neigh], axis=1)  # [1024, 174]
    xsT = np.ascontiguousarray(x_in.T)  # [174, 1024]

    lt_hi = _pack_kt(np.ascontiguousarray(L.T).astype(BF))
    dt_hi = _pack_kt(np.ascontiguousarray(D.T).astype(BF))

    ids_p = np.ascontiguousarray(
        aids.astype(np.int32).reshape(NQ, 128).T)  # [128p, 8q]

    negA1_full = -np.exp(A1)  # [128]
    negA2_full = -np.exp(A2)

    common = {
        "lt_hi": lt_hi, "dt_hi": dt_hi,
        "xsT_a": xsT[:128].astype(BF),
        "xsT_b": np.ascontiguousarray(xsT[128:]).astype(BF),
        "wtune_a": W_tune[:128].astype(BF),
        "wtune_b": np.ascontiguousarray(W_tune[128:]).astype(BF),
    }
    actb = np.array(
        [0.5 * np.log(H)]
        + [np.log(w) for w in T_W]
        + [np.log(w * t) for w, t in zip(T_W, T_NODES)]
        + [np.log(w * t * t) for w, t in zip(T_W, T_NODES)],
        np.float32)  # [25]

    in_maps = []
    for c in range(NCORES):
        hs = slice(c * HS, (c + 1) * HS)
        wb1c = np.concatenate([W_B1[:, hs], W_dt], axis=1).astype(BF)
        wb2c = np.concatenate([W_B2[:, hs], W_dt], axis=1).astype(BF)
        # packA: 0 btune | 1-2 rms | 3-36 bbc1,bbc2 | 37-68 negA1,negA2 |
        # 69-93 actb | 94-101 ids bits | 102-229 identity
        packa = np.zeros((128, 230), np.float32)
        packa[:, 0] = b_tune
        packa[:, 1] = rms1
        packa[:, 2] = rms2
        packa[:, 3:20] = np.concatenate([b_B1[hs], b_dt])
        packa[:, 20:37] = np.concatenate([b_B2[hs], b_dt])
        packa[:, 37:53] = negA1_full[hs]
        packa[:, 53:69] = negA2_full[hs]
        packa[:, 69:94] = actb
        packa[:, 94:102] = ids_p.view(np.float32)
        packa[:, 102:230] = np.eye(128, dtype=np.float32)
        packb = np.concatenate(
            [wb1c, wb2c, np.ones((128, 1), BF)], axis=1)
        in_maps.append({
            **common,
            "packA": packa, "packB": np.ascontiguousarray(packb),
            "mc": np.ascontiguousarray(
                np.concatenate([m1[:, hs], m2[:, hs]], axis=1)),
        })

    nc = build_bass()
    res = run_bass_kernel_spmd(nc, in_maps, core_ids=list(range(NCORES)),
                               trace=trace, **(trace_kwargs or {}))

    out = np.zeros((2, NA, H), np.float32)
    for c in range(NCORES):
        hs = slice(c * HS, (c + 1) * HS)
        # packed [128p, 8q, 16h] -> [1024, 16]
        out[0][:, hs] = res.results[c]["c1o"].transpose(1, 0, 2).reshape(NA, HS)
        out[1][:, hs] = res.results[c]["c2o"].transpose(1, 0, 2).reshape(NA, HS)
    return out, res



# revision 70
# speedup vs baseline: 1.2547x; 1.1759x over previous
"""Trainium2 Bass kernel for nn_MemoryModel (scatter_memory, 8 cores).

Math (per stage): the 8-point Gauss-Legendre quadrature over matrix
polynomials collapses algebraically:

  LHS_k = I - REG*t_k*D + REG^2*(t_k*D@L + t_k^2/2*D@D)      (D=delta_L, L=L_agg)
  integral = sum_k (LHS_k @ (w_k*V)) * exp(dA*t_k)
           = V*S0 - REG*U*S1 + REG^2*P*S1 + REG^2/2*Q*S2
  with V = X - REG*(L@X),  U = D@V, W1 = L@V, P = D@W1, Q = D@U
  and moments S_j = sum_k w_k t_k^j exp(dA t_k)   (elementwise [n,H])
  As_bar @ M = M - REG*(D@M) + REG^2*(D@(L@M)) + REG^2/2*(D@(D@M))

So each stage costs 9 matmuls of [1024,1024]@[1024,16] per core instead of
nine n^3 products; no n^3 matmul anywhere.

Sharding: H=128 is column-sharded 8 ways (16 cols/core). The [1024,1024]
operators (as transposed, k-tile-packed bf16 hi/lo splits) are replicated;
the per-node small pipeline runs in "transposed land" (H on partitions)
replicated on every core; heavy chains run per-core on the 16-column shard
in node-packed layout [128p, 8q, 16h] (node = 128q+p). The memory tables
m1/m2 are column-sharded [100000,16] per core and gathered on-device with
indirect DMA. One AllGather ([16,1024] -> [128,1024]) carries stage-1
output c1^T to all cores for stage 2.
"""
import os
import sys

import numpy as np

for _p in ("/opt/trn_rl_repo", "/root/.axon_site/_ro/trn_rl_repo"):
    if os.path.isdir(_p) and _p not in sys.path:
        sys.path.insert(0, _p)

import ml_dtypes  # noqa: E402
import concourse.bass as bass  # noqa: E402
import concourse.bacc as bacc  # noqa: E402
import concourse.mybir as mybir  # noqa: E402
import concourse.tile as tile  # noqa: E402
from concourse.bass_utils import run_bass_kernel_spmd  # noqa: E402

F32 = mybir.dt.float32
BF16 = mybir.dt.bfloat16
I32 = mybir.dt.int32
AF = mybir.ActivationFunctionType
OP = mybir.AluOpType
BF = ml_dtypes.bfloat16

NA, H, DIN, E, NN, ED = 1024, 128, 172, 256, 100000, 1
KD = DIN + 2 * ED  # 174
REG = 0.1
REG2 = REG * REG
NCORES = 8
HS = 16  # H columns per core
NQ = 8  # node tiles (1024/128)

_gl_nodes = [-0.1834346424956498, -0.525532409916329, -0.7966664774136267,
             -0.9602898564975363, 0.1834346424956498, 0.525532409916329,
             0.7966664774136267, 0.9602898564975363]
_gl_w = [0.362683783378362, 0.3137066458778873, 0.2223810344533745,
         0.1012285362903763] * 2
T_NODES = [0.5 * (x + 1.0) for x in _gl_nodes]
T_W = [0.5 * w for w in _gl_w]

SPLIT_FIRST = False  # hi/lo bf16 split for the first-order passes (L1, D1)

_BUILD_CACHE = {}


def _pin_act_table_set():
    """Restrict walrus's ACT-table choice to natural_log_exp_and_others so
    the kernel's exp/ln mix never ping-pongs table loads (the default
    greedy per-function pick reloads ~10x per run, ~1.3us each)."""
    if os.environ.get("BASS_ACT_ROOT_JSON_PATH"):
        return
    try:
        import glob
        import json
        import tempfile

        import neuronxcc

        pwp = os.path.join(os.path.dirname(neuronxcc.__file__), "pwp",
                           "pwp_bin_trainium")
        info = json.load(open(os.path.join(pwp, "act_info.json")))
        keep_names = ["natural_log_exp_and_others",
                      "gelu_apprx_tanh_and_others"]
        keep = [s for s in info["act_func_sets"] if s["name"] in keep_names]
        keep.sort(key=lambda s: keep_names.index(s["name"]))
        if len(keep) != len(keep_names):
            return
        d = tempfile.mkdtemp(prefix="act_root_")
        for f in glob.glob(os.path.join(pwp, "*")):
            dst = os.path.join(d, os.path.basename(f))
            if not os.path.exists(dst):
                os.symlink(f, dst)
        out = dict(info)
        out["act_func_sets"] = keep
        patched = os.path.join(d, "act_info.json")
        os.unlink(patched)
        with open(patched, "w") as fh:
            json.dump(out, fh)
        # bacc pre-places the table loads itself (set id = index into
        # act_info.json) - patch its table lookup to match the trimmed json
        import concourse.hw_specs as hw_specs

        tables = {
            s["name"]: {AF.from_pwp(v) for v in s["act"].keys()} for s in keep
        }

        def _tables(arch, _t=tables):
            return _t

        hw_specs.get_activation_tables = _tables
        bacc.get_activation_tables = _tables
        os.environ["BASS_ACT_ROOT_JSON_PATH"] = patched
    except Exception:
        pass


def _heavy_pass(nc, psum, op_parts, rhs_tile, ncols, out_cb, rhs_cols=None):
    """out = Op @ X for a grouped rhs: Op given as list of k-packed lhsT
    sbuf tiles [128, 8, 1024] (bf16 hi [+ lo]); rhs_tile [128, 8, ncols]
    bf16. Calls out_cb(q, psum_tile[128, ncols]) per node tile q."""
    for q in range(NQ):
        ps = psum.tile([128, ncols], F32, tag="hv")
        n_mm = len(op_parts) * NQ
        i = 0
        for part in op_parts:
            for k in range(NQ):
                rhs = rhs_tile[:, k, :ncols] if rhs_cols is None else rhs_cols(k)
                nc.tensor.matmul(
                    ps[:],
                    lhsT=part[:, k, q * 128:(q + 1) * 128],
                    rhs=rhs,
                    start=(i == 0),
                    stop=(i == n_mm - 1),
                )
                i += 1
        out_cb(q, ps)


def build_bass():
    if "nc" in _BUILD_CACHE:
        return _BUILD_CACHE["nc"]
    _pin_act_table_set()
    nc = bacc.Bacc("TRN2", target_bir_lowering=False, debug=False,
                   num_devices=NCORES)
    dp = nc.declare_dram_parameter

    # --- kernel inputs (per-core host-prepped) ---
    # All small per-core constants ride ONE packed f32 param (the DMA
    # engines are descriptor-rate-bound at ~120ns/descriptor: each separate
    # [128,*] load costs 128 descriptors regardless of size).
    # packA cols: 0 btune | 1-2 rms1,rms2 | 3-19 bbc1 | 20-36 bbc2 |
    #   37-52 negA1 | 53-68 negA2 | 69-93 actbias | 94-101 ids(i32 bits) |
    #   102-229 identity
    PK_BT, PK_RMS, PK_BBC, PK_NEGA, PK_ACTB, PK_IDS, PK_ID = \
        0, 1, 3, 37, 69, 94, 102
    PKA_N = 230
    packA = dp("packA", [128, PKA_N], F32, isOutput=False)
    # packB cols (bf16): 0-16 wb1 | 17-33 wb2 | 34 ones
    packB = dp("packB", [128, 35], BF16, isOutput=False)
    lt_hi = dp("lt_hi", [128, NQ * 1024], BF16, isOutput=False)
    dt_hi = dp("dt_hi", [128, NQ * 1024], BF16, isOutput=False)
    xsT_a = dp("xsT_a", [128, 1024], BF16, isOutput=False)
    xsT_b = dp("xsT_b", [KD - 128, 1024], BF16, isOutput=False)
    wtune_a = dp("wtune_a", [128, 128], BF16, isOutput=False)
    wtune_b = dp("wtune_b", [KD - 128, 128], BF16, isOutput=False)
    mc = dp("mc", [NN, 2 * HS], F32, isOutput=False)  # [m1 hs | m2 hs]

    c1o = dp("c1o", [128, NQ, HS], F32, isOutput=True)
    c2o = dp("c2o", [128, NQ, HS], F32, isOutput=True)

    # collective bounce buffers (fp16 payload: halves collective bytes)
    F16 = mybir.dt.float16
    ag_in = nc.dram_tensor("ag_in", [HS, 1024], F16)
    ag_out = nc.dram_tensor("ag_out", [128, 1024], F16, addr_space="Shared")
    # dummy pre-warm collective target: absorbs the ~25us cross-core
    # rendezvous cost under stage-1 compute so the real AllGather is cheap
    dum_in = nc.dram_tensor("dum_in", [1, 128], F32)
    dum_out = nc.dram_tensor("dum_out", [NCORES, 128], F32, addr_space="Shared")

    with tile.TileContext(nc) as tc:
        with tc.tile_pool(name="const", bufs=1) as cst, \
             tc.tile_pool(name="work", bufs=1) as wk, \
             tc.tile_pool(name="psum", bufs=4, space="PSUM") as psum, \
             tc.tile_pool(name="psmall", bufs=2, space="PSUM") as psmall, \
             tc.tile_pool(name="ptrp", bufs=2, space="PSUM") as ptrp:

            # ---------- constant loads ----------
            pack_sb = cst.tile([128, PKA_N], F32, tag="packA")
            packb_sb = cst.tile([128, 35], BF16, tag="packB")
            xsT_a_sb = cst.tile([128, 1024], BF16, tag="xsTa")
            xsT_b_sb = cst.tile([KD - 128, 1024], BF16, tag="xsTb")
            wtune_a_sb = cst.tile([128, 128], BF16, tag="wta")
            wtune_b_sb = cst.tile([KD - 128, 128], BF16, tag="wtb")

            nc.sync.dma_start(out=pack_sb[:], in_=packA[:])

            # pre-warm the collective path: tiny dummy AllGather issued at
            # start so the cross-core rendezvous (~12us+) hides under
            # stage-1 compute and the real AllGather's algo starts promptly.
            # dram->dram feed: no SBUF dependency, fires as the first DMA.
            nc.sync.dma_start(out=dum_in[:], in_=packA[0:1, 0:128])
            nc.gpsimd.collective_compute(
                "AllGather", OP.bypass,
                replica_groups=[list(range(NCORES))],
                ins=[dum_in[:]], outs=[dum_out[:]],
            )

            # memory-table gathers (early; both stages' rows in one pass
            # over the concatenated [NN, 32] table)
            mg_both = wk.tile([128, NQ, 2 * HS], F32, tag="mgb")
            for q in range(NQ):
                nc.gpsimd.indirect_dma_start(
                    out=mg_both[:, q, :],
                    out_offset=None,
                    in_=mc[:],
                    in_offset=bass.IndirectOffsetOnAxis(
                        ap=pack_sb[:, PK_IDS + q:PK_IDS + q + 1].bitcast(I32),
                        axis=0),
                )

            nc.sync.dma_start(out=packb_sb[:], in_=packB[:])
            nc.sync.dma_start(out=xsT_a_sb[:], in_=xsT_a[:])
            nc.sync.dma_start(out=xsT_b_sb[:], in_=xsT_b[:])
            nc.sync.dma_start(out=wtune_a_sb[:], in_=wtune_a[:])
            nc.sync.dma_start(out=wtune_b_sb[:], in_=wtune_b[:])

            # [128,1,*] views of packed consts for middle-dim broadcasts
            bbc_sb = [cst.tile([128, 1, HS + 1], F32, tag=f"bbc{s}", name=f"bbc_sb{s}") for s in range(2)]
            negA_t = [cst.tile([128, 1, HS], F32, tag=f"negA{s}", name=f"negA_t{s}") for s in range(2)]
            for s in range(2):
                nc.vector.tensor_copy(
                    out=bbc_sb[s][:, 0, :],
                    in_=pack_sb[:, PK_BBC + 17 * s:PK_BBC + 17 * (s + 1)])
                nc.vector.tensor_copy(
                    out=negA_t[s][:, 0, :],
                    in_=pack_sb[:, PK_NEGA + HS * s:PK_NEGA + HS * (s + 1)])
            wb_sb = [packb_sb[:, 17 * s:17 * (s + 1)] for s in range(2)]
            ones_ap = packb_sb[:, 34:35]
            ident_ap = pack_sb[:, PK_ID:PK_ID + 128]

            # operator loads (big; overlap with small pipeline)
            lt_sb = [cst.tile([128, NQ, 1024], BF16, tag="lt_hi", name="lt_hi_sb")]
            dt_sb = [cst.tile([128, NQ, 1024], BF16, tag="dt_hi", name="dt_hi_sb")]
            nc.sync.dma_start(out=lt_sb[0][:], in_=lt_hi[:])
            nc.sync.dma_start(out=dt_sb[0][:], in_=dt_hi[:])

            # zt^T = W_tune^T @ x_in^T + b_tune   [128 H, 1024 nodes] f32
            ztT = wk.tile([128, 1024], F32, tag="ztT")
            for hhalf in range(2):
                ps = psmall.tile([128, 512], F32, tag="sp")
                cols = slice(hhalf * 512, (hhalf + 1) * 512)
                nc.tensor.matmul(ps[:], lhsT=wtune_a_sb[:],
                                 rhs=xsT_a_sb[:, cols], start=True, stop=False)
                nc.tensor.matmul(ps[:], lhsT=wtune_b_sb[:],
                                 rhs=xsT_b_sb[:, cols], start=False, stop=True)
                nc.vector.tensor_scalar(out=ztT[:, cols], in0=ps[:],
                                        scalar1=pack_sb[:, PK_BT:PK_BT + 1],
                                        scalar2=None, op0=OP.add)

            c1T_full = wk.tile([128, 1024], mybir.dt.float16, tag="c1T_full")
            u2T = wk.tile([128, 1024], F32, tag="u2T")
            gtmp = wk.tile([128, 1024], F32, tag="gtmp")

            couts = (c1o, c2o)

            for s in range(2):  # the two SSM stages
                if s == 0:
                    base = ztT
                else:
                    # u2 = zt + gelu(c1) via the HW tanh-approx gelu table,
                    # split in node-halves so DVE's add overlaps ACT's gelu
                    for h2 in range(2):
                        nco = slice(h2 * 512, (h2 + 1) * 512)
                        nc.scalar.activation(gtmp[:, nco], c1T_full[:, nco],
                                             AF.Gelu_apprx_tanh)
                        nc.vector.tensor_tensor(out=u2T[:, nco],
                                                in0=ztT[:, nco],
                                                in1=gtmp[:, nco], op=OP.add)
                    base = u2T

                # small pipeline in two node-halves: half 0's PE/ACT work
                # overlaps half 1's DVE work and vice versa
                baseS = wk.tile([128, 1024], BF16, tag=f"baseS{s}")
                sq = wk.tile([128, 1024], BF16, tag=f"sq{s}")
                lnss = wk.tile([128, NQ, 1], F32, tag=f"lnss{s}")
                rinv = wk.tile([128, NQ, 1], F32, tag=f"rinv{s}")
                BD = wk.tile([128, NQ, HS + 1], F32, tag=f"BD{s}")
                esp = wk.tile([128, NQ, 1], F32, tag=f"esp{s}")
                deltap = wk.tile([128, NQ, 1], F32, tag=f"deltap{s}")
                R0 = wk.tile([128, NQ, 2 * HS], BF16, tag=f"R0{s}")
                dA = wk.tile([128, NQ, HS], F32, tag=f"dA{s}")
                At = wk.tile([128, NQ, HS], F32, tag=f"At{s}")
                Mf = wk.tile([128, NQ, HS], F32, tag=f"Mf{s}")
                NH = NQ // 2
                for h2 in range(2):
                    qs = slice(h2 * NH, (h2 + 1) * NH)
                    nco = slice(h2 * 512, (h2 + 1) * 512)
                    # scaled bf16 lhsT for the B/delta matmuls + squares
                    nc.vector.tensor_scalar(
                        out=baseS[:, nco], in0=base[:, nco],
                        scalar1=pack_sb[:, PK_RMS + s:PK_RMS + s + 1],
                        scalar2=None, op0=OP.mult)
                    nc.vector.tensor_tensor(out=sq[:, nco], in0=base[:, nco],
                                            in1=base[:, nco], op=OP.mult)

                    # ss[p,q] = sum_H zt^2 ; rinv = 1/sqrt(ss/H) via exp/ln
                    ssp = psmall.tile([128, NH, 1], F32, tag="sp")
                    for qi in range(NH):
                        q = h2 * NH + qi
                        nc.tensor.matmul(ssp[:, qi, :],
                                         lhsT=sq[:, q * 128:(q + 1) * 128],
                                         rhs=ones_ap, start=True, stop=True)
                    nc.scalar.activation(lnss[:, qs, :], ssp[:], AF.Ln)
                    nc.scalar.activation(rinv[:, qs, :], lnss[:, qs, :],
                                         AF.Exp, scale=-0.5,
                                         bias=pack_sb[:, PK_ACTB:PK_ACTB + 1])

                    # B/delta matmuls + normalization fold
                    psb = psmall.tile([128, NH, HS + 1], F32, tag="sp")
                    for qi in range(NH):
                        q = h2 * NH + qi
                        nc.tensor.matmul(psb[:, qi, :],
                                         lhsT=baseS[:, q * 128:(q + 1) * 128],
                                         rhs=wb_sb[s], start=True, stop=True)
                    nc.vector.tensor_tensor(
                        out=BD[:, qs, :], in0=psb[:],
                        in1=rinv[:, qs, :].to_broadcast([128, NH, HS + 1]),
                        op=OP.mult)
                    nc.vector.tensor_tensor(
                        out=BD[:, qs, :], in0=BD[:, qs, :],
                        in1=bbc_sb[s][:].to_broadcast([128, NH, HS + 1]),
                        op=OP.add)

                    # delta = softplus = ln(1+exp(x)); +1 rides the Ln bias
                    nc.scalar.activation(esp[:, qs, :], BD[:, qs, HS:HS + 1],
                                         AF.Exp)
                    nc.scalar.activation(deltap[:, qs, :], esp[:, qs, :],
                                         AF.Ln, bias=1.0)

                    # X = B*delta (bf16 into R0); dA = delta*negA;
                    # At = exp(dA); M = m_gather*At
                    nc.vector.tensor_tensor(
                        out=R0[:, qs, 0:HS], in0=BD[:, qs, 0:HS],
                        in1=deltap[:, qs, :].to_broadcast([128, NH, HS]),
                        op=OP.mult)
                    nc.vector.tensor_tensor(
                        out=dA[:, qs, :],
                        in0=deltap[:, qs, :].to_broadcast([128, NH, HS]),
                        in1=negA_t[s][:].to_broadcast([128, NH, HS]),
                        op=OP.mult)
                    nc.scalar.activation(At[:, qs, :], dA[:, qs, :], AF.Exp)
                    nc.vector.tensor_tensor(
                        out=Mf[:, qs, :],
                        in0=mg_both[:, qs, s * HS:(s + 1) * HS],
                        in1=At[:, qs, :], op=OP.mult)
                    nc.vector.tensor_copy(out=R0[:, qs, HS:2 * HS],
                                          in_=Mf[:, qs, :])

                # moments S_j[p,q,h] = sum_k w_k t_k^j exp(dA t_k). The
                # t_k^j factors ride the exp bias (ln(w_k t_k^j) columns of
                # actbias), so accumulation is pure adds on the otherwise
                # idle GpSimd engine. Chunks are interleaved between heavy
                # passes (see below) to fill ACT idle time without delaying
                # pass callbacks.
                Smom = [wk.tile([128, NQ, HS], BF16, tag=f"S{j}{s}",
                                name=f"S{j}{s}") for j in range(3)]

                def emit_moments(j, s=s, Smom=Smom, dA=dA):
                    wEs = []
                    for k in range(8):
                        wE = wk.tile([128, NQ, HS], BF16, tag=f"wE{s}_{j}_{k}",
                                     name=f"wE{s}_{j}_{k}")
                        nc.scalar.activation(
                            wE[:], dA[:], AF.Exp, scale=float(T_NODES[k]),
                            bias=pack_sb[:, PK_ACTB + 1 + 8 * j + k:
                                         PK_ACTB + 2 + 8 * j + k])
                        wEs.append(wE)
                    # pairwise tree add on GpSimd
                    for a, b in ((0, 1), (2, 3), (4, 5), (6, 7)):
                        nc.gpsimd.tensor_tensor(out=wEs[a][:], in0=wEs[a][:],
                                                in1=wEs[b][:], op=OP.add)
                    for a, b in ((0, 2), (4, 6)):
                        nc.gpsimd.tensor_tensor(out=wEs[a][:], in0=wEs[a][:],
                                                in1=wEs[b][:], op=OP.add)
                    nc.gpsimd.tensor_tensor(out=Smom[j][:], in0=wEs[0][:],
                                            in1=wEs[4][:], op=OP.add)

                # ---- heavy pass L1: L @ [X | M] -> LX, Y1 ----
                R1 = wk.tile([128, NQ, 3 * HS], BF16, tag=f"R1{s}")  # [V|M|Y1]
                nc.vector.tensor_copy(out=R1[:, :, HS:2 * HS],
                                      in_=R0[:, :, HS:2 * HS])

                def l1_cb(q, ps, s=s, R1=R1, R0=R0):
                    # V = X - REG*LX  (bf16 into R1) ; Y1 = psum[:,16:32]
                    nc.vector.scalar_tensor_tensor(
                        out=R1[:, q, 0:HS], in0=ps[:, 0:HS], scalar=-REG,
                        in1=R0[:, q, 0:HS], op0=OP.mult, op1=OP.add)
                    nc.scalar.activation(R1[:, q, 2 * HS:3 * HS],
                                         ps[:, HS:2 * HS], AF.Copy)

                _heavy_pass(nc, psum, lt_sb, R0, 2 * HS, l1_cb)

                # ---- heavy pass D1: D @ [V | M | Y1] -> U, UM, T1 ----
                R2 = wk.tile([128, NQ, 3 * HS], BF16, tag=f"R2{s}")  # [W1|U|UM]
                T1b = wk.tile([128, NQ, HS], BF16, tag=f"T1b{s}")

                def d1_cb(q, ps, R2=R2, T1b=T1b):
                    # spread across DVE/Pool: ACT is the stage-2 bottleneck
                    nc.vector.tensor_copy(out=R2[:, q, HS:3 * HS],
                                          in_=ps[:, 0:2 * HS])
                    nc.scalar.activation(T1b[:, q, :], ps[:, 2 * HS:3 * HS],
                                         AF.Copy)

                _heavy_pass(nc, psum, dt_sb, R1, 3 * HS, d1_cb)
                emit_moments(0)

                # ---- heavy pass L2: L @ V -> W1 ----
                def l2_cb(q, ps, R2=R2):
                    nc.scalar.activation(R2[:, q, 0:HS], ps[:, 0:HS], AF.Copy)

                _heavy_pass(nc, psum, lt_sb[:1], R1, HS, l2_cb)
                emit_moments(1)

                # ---- heavy pass D2: D @ [W1 | U | UM] -> P, Q, T2 ----
                OUT2 = wk.tile([128, NQ, 3 * HS], BF16, tag=f"OUT2{s}")

                def d2_cb(q, ps, OUT2=OUT2):
                    nc.vector.tensor_copy(out=OUT2[:, q, :], in_=ps[:])

                _heavy_pass(nc, psum, dt_sb[:1], R2, 3 * HS, d2_cb)
                emit_moments(2)

                # ---- combine ----
                # S-products on GpSimd in parallel with the M-term chain on
                # DVE; DVE then folds everything.
                acc = wk.tile([128, NQ, HS], F32, tag=f"acc{s}")
                pV = wk.tile([128, NQ, HS], F32, tag=f"pV{s}")
                pU = wk.tile([128, NQ, HS], F32, tag=f"pU{s}")
                pP = wk.tile([128, NQ, HS], F32, tag=f"pP{s}")
                pQ = wk.tile([128, NQ, HS], F32, tag=f"pQ{s}")
                nc.gpsimd.tensor_tensor(out=pV[:], in0=R1[:, :, 0:HS],
                                        in1=Smom[0][:], op=OP.mult)
                nc.gpsimd.tensor_tensor(out=pU[:], in0=R2[:, :, HS:2 * HS],
                                        in1=Smom[1][:], op=OP.mult)
                nc.gpsimd.tensor_tensor(out=pP[:], in0=OUT2[:, :, 0:HS],
                                        in1=Smom[1][:], op=OP.mult)
                nc.gpsimd.tensor_tensor(out=pQ[:], in0=OUT2[:, :, HS:2 * HS],
                                        in1=Smom[2][:], op=OP.mult)
                # acc = M - REG*UM
                nc.vector.scalar_tensor_tensor(
                    out=acc[:], in0=R2[:, :, 2 * HS:3 * HS], scalar=-REG,
                    in1=Mf[:], op0=OP.mult, op1=OP.add)
                # + REG^2*T1
                nc.vector.scalar_tensor_tensor(
                    out=acc[:], in0=T1b[:], scalar=REG2, in1=acc[:],
                    op0=OP.mult, op1=OP.add)
                # + REG^2/2*T2
                nc.vector.scalar_tensor_tensor(
                    out=acc[:], in0=OUT2[:, :, 2 * HS:3 * HS], scalar=REG2 / 2,
                    in1=acc[:], op0=OP.mult, op1=OP.add)
                # + V*S0
                nc.vector.tensor_tensor(out=acc[:], in0=acc[:], in1=pV[:],
                                        op=OP.add)
                # - REG*U*S1
                nc.vector.scalar_tensor_tensor(
                    out=acc[:], in0=pU[:], scalar=-REG, in1=acc[:],
                    op0=OP.mult, op1=OP.add)
                # + REG^2*P*S1
                nc.vector.scalar_tensor_tensor(
                    out=acc[:], in0=pP[:], scalar=REG2, in1=acc[:],
                    op0=OP.mult, op1=OP.add)
                # + REG^2/2*Q*S2
                nc.vector.scalar_tensor_tensor(
                    out=acc[:], in0=pQ[:], scalar=REG2 / 2, in1=acc[:],
                    op0=OP.mult, op1=OP.add)

                # write output shard
                nc.sync.dma_start(out=couts[s][:], in_=acc[:])

                if s == 0:
                    # transpose c1 shard to [16,1024], AllGather to c1T_full
                    c1Ts = wk.tile([HS, 1024], mybir.dt.float16, tag="c1Ts")
                    for q in range(NQ):
                        pst = ptrp.tile([HS, 128], F32, tag="trp")
                        nc.tensor.transpose(pst[:], acc[:, q, :], ident_ap)
                        nc.vector.tensor_copy(
                            out=c1Ts[:, q * 128:(q + 1) * 128], in_=pst[:])
                    nc.sync.dma_start(out=ag_in[:], in_=c1Ts[:])
                    nc.gpsimd.collective_compute(
                        "AllGather", OP.bypass,
                        replica_groups=[list(range(NCORES))],
                        ins=[ag_in[:]], outs=[ag_out[:]],
                    )
                    nc.sync.dma_start(out=c1T_full[:], in_=ag_out[:])

    nc.compile()
    _BUILD_CACHE["nc"] = nc
    return nc


def _split_bf16(a):
    hi = a.astype(BF)
    lo = (a - hi.astype(np.float32)).astype(BF)
    return hi, lo


def _pack_kt(a_T):
    """[1024, 1024] (k-major rows) -> [128, 8*1024] partition-packed bf16 pair."""
    r = a_T.reshape(NQ, 128, 1024).transpose(1, 0, 2).reshape(128, NQ * 1024)
    return r


def kernel(**inputs):
    out, _ = _run(inputs, trace=False)
    return out


def _run(inputs, trace=False, trace_kwargs=None):
    inp = {k: np.asarray(v) for k, v in inputs.items()}
    L = inp["L_agg"].astype(np.float32)
    D = inp["delta_L_agg"].astype(np.float32)
    x_sub = inp["x_sub"].astype(np.float32)
    m1 = inp["m1_vec"].astype(np.float32)
    m2 = inp["m2_vec"].astype(np.float32)
    names = inp["names_table"].astype(np.float32)
    rms1 = inp["rms1_scale"].astype(np.float32)
    rms2 = inp["rms2_scale"].astype(np.float32)
    W_tune = inp["W_tune"].astype(np.float32)
    b_tune = inp["b_tune"].astype(np.float32)
    W_B1 = inp["W_B1"].astype(np.float32)
    b_B1 = inp["b_B1"].astype(np.float32)
    W_B2 = inp["W_B2"].astype(np.float32)
    b_B2 = inp["b_B2"].astype(np.float32)
    W_dt = inp["W_dt"].astype(np.float32)
    b_dt = inp["b_dt"].astype(np.float32)
    A1 = inp["A_log_1"].astype(np.float32)
    A2 = inp["A_log_2"].astype(np.float32)
    tsrc = np.asarray(inp["target_src"]).astype(np.int64)
    tdst = np.asarray(inp["target_dst"]).astype(np.int64)
    aids = np.asarray(inp["active_input_ids"]).astype(np.int64)

    # x_in = [x_sub | neigh]; the names_table neighbor embedding (ED=1)
    neigh = np.zeros((NA, 2 * ED), np.float32)
    neigh[:E, :ED] = names[tsrc]
    neigh[:E, ED:] = names[tdst]
    neigh[E:2 * E, :ED] = names[tdst]
    neigh[E:2 * E, ED:] = names[tsrc]
    x_in = np.concatenate([x_sub, neigh], axis=1)  # [1024, 174]
    xsT = np.ascontiguousarray(x_in.T)  # [174, 1024]

    lt_hi = _pack_kt(np.ascontiguousarray(L.T).astype(BF))
    dt_hi = _pack_kt(np.ascontiguousarray(D.T).astype(BF))

    ids_p = np.ascontiguousarray(
        aids.astype(np.int32).reshape(NQ, 128).T)  # [128p, 8q]

    negA1_full = -np.exp(A1)  # [128]
    negA2_full = -np.exp(A2)

    common = {
        "lt_hi": lt_hi, "dt_hi": dt_hi,
        "xsT_a": xsT[:128].astype(BF),
        "xsT_b": np.ascontiguousarray(xsT[128:]).astype(BF),
        "wtune_a": W_tune[:128].astype(BF),
        "wtune_b": np.ascontiguousarray(W_tune[128:]).astype(BF),
    }
    actb = np.array(
        [0.5 * np.log(H)]
        + [np.log(w) for w in T_W]
        + [np.log(w * t) for w, t in zip(T_W, T_NODES)]
        + [np.log(w * t * t) for w, t in zip(T_W, T_NODES)],
        np.float32)  # [25]

    in_maps = []
    for c in range(NCORES):
        hs = slice(c * HS, (c + 1) * HS)
        wb1c = np.concatenate([W_B1[:, hs], W_dt], axis=1).astype(BF)
        wb2c = np.concatenate([W_B2[:, hs], W_dt], axis=1).astype(BF)
        # packA: 0 btune | 1-2 rms | 3-36 bbc1,bbc2 | 37-68 negA1,negA2 |
        # 69-93 actb | 94-101 ids bits | 102-229 identity
        packa = np.zeros((128, 230), np.float32)
        packa[:, 0] = b_tune
        packa[:, 1] = rms1
        packa[:, 2] = rms2
        packa[:, 3:20] = np.concatenate([b_B1[hs], b_dt])
        packa[:, 20:37] = np.concatenate([b_B2[hs], b_dt])
        packa[:, 37:53] = negA1_full[hs]
        packa[:, 53:69] = negA2_full[hs]
        packa[:, 69:94] = actb
        packa[:, 94:102] = ids_p.view(np.float32)
        packa[:, 102:230] = np.eye(128, dtype=np.float32)
        packb = np.concatenate(
            [wb1c, wb2c, np.ones((128, 1), BF)], axis=1)
        in_maps.append({
            **common,
            "packA": packa, "packB": np.ascontiguousarray(packb),
            "mc": np.ascontiguousarray(
                np.concatenate([m1[:, hs], m2[:, hs]], axis=1)),
        })

    nc = build_bass()
    res = run_bass_kernel_spmd(nc, in_maps, core_ids=list(range(NCORES)),
                               trace=trace, **(trace_kwargs or {}))

    out = np.zeros((2, NA, H), np.float32)
    for c in range(NCORES):
        hs = slice(c * HS, (c + 1) * HS)
        # packed [128p, 8q, 16h] -> [1024, 16]
        out[0][:, hs] = res.results[c]["c1o"].transpose(1, 0, 2).reshape(NA, HS)
        out[1][:, hs] = res.results[c]["c2o"].transpose(1, 0, 2).reshape(NA, HS)
    return out, res

